# revision 48
# baseline (speedup 1.0000x reference)
"""MinkowskiGlobalPooling (average=True) segment-mean kernel for 8 trn2 cores.

Full inputs in, full output out. Strategy (fp8 + transposed matmul):
  - rows are sharded across 8 cores (500k rows each), laid out per core as
    128 SBUF partitions x R=3920 rows (tail rows padded, local idx=255),
  - feats are quantized host-side to float8_e3m4 (e3m4 keeps the pooled
    mean's rel-err at ~1.4e-2, under the 2e-2 gate); the stream is 64 fp8
    channels per row in a chunk-major layout,
  - batch_idx is sorted, so each core only sees a window of <=8 distinct
    batches; host rebases idx to a local 0..7 window (u8 sideband, stored
    even/odd-position split per mask block),
  - matmul is transposed: the [128,128] stationary operand is a PAIR of
    row-positions' feats (2x64 channels) so the compiler's Fast Weight
    Load kicks in, and the moving operand is the pair's 16 one-hot mask
    columns built on DVE via tensor_scalar is_equal.
    psum[0:64,0:8] accumulates even-position sums^T, psum[64:128,8:16]
    odd-position sums^T; the two cross quadrants are ignored,
  - counts come from a host-side bincount (exact integers either way),
  - the whole stream rides ONE HWDGE queue (scalar) in fine 196-row
    chunks with deep buffering: a single in-order queue avoids the
    multi-queue phase-locking raggedness and benches fastest/stablest,
  - host folds the per-core [128,16] psums into the global [32,64] and
    divides by counts.
"""

import numpy as np


def _ensure_import_path():
    try:
        import concourse.bass  # noqa: F401
    except ImportError:
        import sys

        for p in ("/opt/trn_rl_repo", "/root/.axon_site/_ro/trn_rl_repo"):
            if p not in sys.path:
                sys.path.insert(0, p)


N_CORES = 8
B = 32  # global batches
W = 8  # local batch window per core (sorted batch_idx => width <= 8)
C = 64  # channels
N_TOTAL = 4_000_000
N_CORE = N_TOTAL // N_CORES  # 500_000 real rows per core
P = 128  # SBUF partitions
R = 3920  # rows per partition (128*3920 = 501_760 >= 500_000; tail is padding)
# chunk sizes (all even): fine-grained chunks keep the DMA ring uniformly
# stocked; small lead-in chunks shorten pipeline fill
SCHEDULE = [98, 98] + [196] * 19
MASK_BLOCKS = [(0, 2), (2, 6), (6, 11), (11, 16), (16, 21)]
assert sum(SCHEDULE) == R
PAD_IDX = 255  # u8 padding local index; matches no batch column
FBUFS = 8  # feats chunk buffers (DMA runahead depth)
MBUFS = 3  # mask block buffers


def build_program(p=P, schedule=None, fbufs=FBUFS, mbufs=MBUFS):
    """Build the per-core Bass program. All cores run the identical program."""
    _ensure_import_path()
    import concourse.mybir as mybir
    from concourse import bacc
    from concourse.tile import TileContext

    f32 = mybir.dt.float32
    f8 = mybir.dt.float8e3
    u8 = mybir.dt.uint8
    if schedule is None:
        schedule = SCHEDULE
    r = sum(schedule)
    n_pair = r // 2

    offs = [0]
    for t in schedule:
        offs.append(offs[-1] + t)
    blocks = []  # (start_pair, n_pairs) per mask block
    chunk_block = [0] * len(schedule)
    for bi_, (c0, c1) in enumerate(MASK_BLOCKS):
        blocks.append((offs[c0] // 2, (offs[c1] - offs[c0]) // 2))
        for c in range(c0, c1):
            chunk_block[c] = bi_
    max_blk = max(n for _, n in blocks)

    nc = bacc.Bacc()
    stream = nc.dram_tensor("stream", [p * r * C], f8, kind="ExternalInput")
    # idxu holds local batch ids, even/odd-position split per mask block
    idxu = nc.dram_tensor("idxu", [p * r], u8, kind="ExternalInput")
    # psum/out keep 128 partitions: out partition dim = lhsT free dim = 128
    out = nc.dram_tensor("out", [2 * C, 2 * W], f32, kind="ExternalOutput")

    with TileContext(nc) as tc:
        with (
            tc.tile_pool(name="const", bufs=1) as cpool,
            tc.tile_pool(name="feats", bufs=fbufs) as fpool,
            tc.tile_pool(name="mask", bufs=mbufs) as mpool,
            tc.tile_pool(name="psum", bufs=1, space="PSUM") as ppool,
            tc.tile_pool(name="outp", bufs=1) as opool,
        ):
            # idx per-partition layout: per mask block [even pairs | odd pairs]
            idx_sb = cpool.tile([p, r], u8)
            idx_dram = idxu[:].rearrange("(p r) -> p r", p=p)
            split = 2 * blocks[0][1]  # bytes covering mask block 0 (even+odd)
            # idx rides the otherwise-idle gpsimd queue, block 0 first
            nc.gpsimd.dma_start(out=idx_sb[:, :split], in_=idx_dram[:, :split])
            nc.gpsimd.dma_start(out=idx_sb[:, split:], in_=idx_dram[:, split:])

            zero_sb = cpool.tile([p, 2 * C], f8)
            nc.vector.memset(zero_sb[:], 0.0)

            psum = ppool.tile([2 * C, 2 * W], f32)
            # zero-valued "start" matmul: all real matmuls then accumulate
            # (start=False), making the result independent of the
            # has_written-clear granularity
            nc.tensor.matmul(
                psum[:],
                lhsT=zero_sb[:],
                rhs=zero_sb[:, : 2 * W],
                start=True,
                stop=False,
            )

            # mask generation (DVE):
            # mask2[p, half*W+j, q] = (idx[p, half, start+q] == j)
            mask_tiles = {}

            def gen_masks(bi_):
                start, npair = blocks[bi_]
                mk = mpool.tile([p, 2 * W * max_blk], f8, tag="mk")
                mkv = mk[:, : 2 * W * npair].rearrange("p (e t) -> p e t", e=2 * W)
                for half in range(2):
                    base = 2 * start + half * npair
                    src = idx_sb[:, base : base + npair]
                    for j in range(W):
                        nc.vector.tensor_scalar(
                            out=mkv[:, half * W + j, :],
                            in0=src,
                            scalar1=float(j),
                            scalar2=None,
                            op0=mybir.AluOpType.is_equal,
                        )
                mask_tiles[bi_] = (mkv, start)

            gen_masks(0)
            k = 0
            for j, t in enumerate(schedule):
                if chunk_block[j] not in mask_tiles:
                    gen_masks(chunk_block[j])
                # look ahead one chunk so mask gen runs a block early
                if j + 1 < len(schedule) and chunk_block[j + 1] not in mask_tiles:
                    gen_masks(chunk_block[j + 1])
                ft = fpool.tile([p, t * C], f8, tag="ft")
                nc.scalar.dma_start(
                    out=ft[:],
                    in_=stream[p * offs[j] * C : p * offs[j + 1] * C].rearrange(
                        "(p x) -> p x", p=p
                    ),
                )
                mkv, mstart = mask_tiles[chunk_block[j]]
                for s in range(t // 2):
                    q = offs[j] // 2 + s  # global pair index
                    nc.tensor.matmul(
                        psum[:],
                        lhsT=ft[:, s * 2 * C : (s + 1) * 2 * C],
                        rhs=mkv[:, :, q - mstart],
                        start=False,
                        stop=(k == n_pair - 1),
                    )
                    k += 1
            out_sb = opool.tile([2 * C, 2 * W], f32)
            nc.vector.tensor_copy(out=out_sb[:], in_=psum[:])
            nc.gpsimd.dma_start(out=out[:, :], in_=out_sb[:])
    nc.finalize()
    return nc


def host_prep(feats, batch_idx):
    """Build per-core input maps (packed fp8 stream + local idx) from full inputs."""
    import ml_dtypes

    feats = np.asarray(feats, dtype=np.float32)
    bi = np.asarray(batch_idx).astype(np.int64)
    n, c = feats.shape
    assert n == N_TOTAL and c == C, (n, c)

    f8 = ml_dtypes.float8_e3m4
    q = feats.astype(f8)
    counts = np.bincount(bi, minlength=B).astype(np.float64)
    offs = np.concatenate([[0], np.cumsum(SCHEDULE)])
    # mask block boundaries in pair units
    blk_bounds = [(offs[c0] // 2, offs[c1] // 2) for c0, c1 in MASK_BLOCKS]

    in_maps = []
    lo_ws = []
    for m in range(N_CORES):
        sl = slice(m * N_CORE, (m + 1) * N_CORE)
        lo, hi = int(bi[m * N_CORE]), int(bi[(m + 1) * N_CORE - 1])
        assert hi - lo + 1 <= W, (m, lo, hi)
        lo_w = min(lo, B - W)
        lo_ws.append(lo_w)

        fpad = np.zeros((P * R, C), dtype=f8)
        fpad[:N_CORE] = q[sl]
        fv = fpad.reshape(P, R, C)
        ipad = np.full(P * R, PAD_IDX, dtype=np.uint8)
        ipad[:N_CORE] = (bi[sl] - lo_w).astype(np.uint8)
        # split even/odd positions per mask block: per partition, each
        # block stores [even pairs | odd pairs]
        iv = ipad.reshape(P, R // 2, 2)
        iparts = []
        for q0, q1 in blk_bounds:
            blk = iv[:, q0:q1]  # [P, npair, 2]
            iparts.append(np.ascontiguousarray(blk.transpose(0, 2, 1)))
        iflat = np.concatenate([x.reshape(P, -1) for x in iparts], axis=1)
        assert iflat.shape == (P, R)

        # chunk-major flat layout: chunk j = [p, t_j, C] contiguous block
        flat = np.empty(P * R * C, dtype=f8)
        pos = 0
        for j, t in enumerate(SCHEDULE):
            blk = fv[:, offs[j] : offs[j] + t]  # [P, t, C]
            flat[pos : pos + blk.size] = blk.reshape(-1)
            pos += blk.size
        in_maps.append({"stream": flat, "idxu": iflat.reshape(-1)})
    return in_maps, (lo_ws, counts)


_CACHED_NC = None


def get_program():
    global _CACHED_NC
    if _CACHED_NC is None:
        _CACHED_NC = build_program()
    return _CACHED_NC


def run_on_cores(in_maps, trace=False):
    _ensure_import_path()
    from concourse.bass_utils import run_bass_kernel_spmd

    nc = get_program()
    res = run_bass_kernel_spmd(nc, in_maps, list(range(N_CORES)), trace=trace)
    return res


def finalize(per_core_outs, aux):
    lo_ws, counts = aux
    sums = np.zeros((B, C), dtype=np.float64)
    for o, lo_w in zip(per_core_outs, lo_ws):
        o = np.asarray(o, dtype=np.float64)  # [128, 16]
        strip = o[:C, :W] + o[C:, W:]  # [64, 8] = sums^T (even + odd)
        sums[lo_w : lo_w + W] += strip.T
    pooled = sums / np.maximum(counts, 1.0)[:, None]
    return pooled.astype(np.float32)


def kernel(feats, batch_idx, num_batches):
    assert int(num_batches) == B
    in_maps, aux = host_prep(feats, batch_idx)
    res = run_on_cores(in_maps)
    return finalize([r["out"] for r in res.results], aux)


# revision 49
# speedup vs baseline: 1.0071x; 1.0071x over previous
"""MinkowskiGlobalPooling (average=True) segment-mean kernel for 8 trn2 cores.

Full inputs in, full output out. Strategy (fp8 + transposed matmul):
  - rows are sharded across 8 cores (500k rows each), laid out per core as
    128 SBUF partitions x R=3920 rows (tail rows padded, local idx=255),
  - feats are quantized host-side to float8_e3m4 (e3m4 keeps the pooled
    mean's rel-err at ~1.4e-2, under the 2e-2 gate); the stream is 64 fp8
    channels per row in a chunk-major layout,
  - batch_idx is sorted, so each core only sees a window of <=8 distinct
    batches; host rebases idx to a local 0..7 window (u8 sideband, stored
    even/odd-position split per mask block),
  - matmul is transposed: the [128,128] stationary operand is a PAIR of
    row-positions' feats (2x64 channels) so the compiler's Fast Weight
    Load kicks in, and the moving operand is the pair's 16 one-hot mask
    columns built on DVE via tensor_scalar is_equal.
    psum[0:64,0:8] accumulates even-position sums^T, psum[64:128,8:16]
    odd-position sums^T; the two cross quadrants are ignored,
  - counts come from a host-side bincount (exact integers either way),
  - the whole stream rides ONE HWDGE queue (scalar) in fine 196-row
    chunks with deep buffering: a single in-order queue avoids the
    multi-queue phase-locking raggedness and benches fastest/stablest,
  - host folds the per-core [128,16] psums into the global [32,64] and
    divides by counts.
"""

import numpy as np


def _ensure_import_path():
    try:
        import concourse.bass  # noqa: F401
    except ImportError:
        import sys

        for p in ("/opt/trn_rl_repo", "/root/.axon_site/_ro/trn_rl_repo"):
            if p not in sys.path:
                sys.path.insert(0, p)


N_CORES = 8
B = 32  # global batches
W = 8  # local batch window per core (sorted batch_idx => width <= 8)
C = 64  # channels
N_TOTAL = 4_000_000
N_CORE = N_TOTAL // N_CORES  # 500_000 real rows per core
P = 128  # SBUF partitions
R = 3920  # rows per partition (128*3920 = 501_760 >= 500_000; tail is padding)
# chunk sizes (all even): fine-grained chunks keep the DMA ring uniformly
# stocked; small lead-in chunks shorten pipeline fill
import os as _os

if _os.environ.get("K_SCHED", "196") == "98":
    SCHEDULE = [98] * 40
    MASK_BLOCKS = [(0, 8), (8, 16), (16, 24), (24, 32), (32, 40)]
else:
    SCHEDULE = [98, 98] + [196] * 19
    MASK_BLOCKS = [(0, 2), (2, 6), (6, 11), (11, 16), (16, 21)]
assert sum(SCHEDULE) == R
PAD_IDX = 255  # u8 padding local index; matches no batch column
FBUFS = int(_os.environ.get("K_FBUFS", "8"))  # feats chunk buffers
MBUFS = int(_os.environ.get("K_MBUFS", "3"))  # mask block buffers


def build_program(p=P, schedule=None, fbufs=FBUFS, mbufs=MBUFS):
    """Build the per-core Bass program. All cores run the identical program."""
    _ensure_import_path()
    import concourse.mybir as mybir
    from concourse import bacc
    from concourse.tile import TileContext

    f32 = mybir.dt.float32
    f8 = mybir.dt.float8e3
    u8 = mybir.dt.uint8
    if schedule is None:
        schedule = SCHEDULE
    r = sum(schedule)
    n_pair = r // 2

    offs = [0]
    for t in schedule:
        offs.append(offs[-1] + t)
    blocks = []  # (start_pair, n_pairs) per mask block
    chunk_block = [0] * len(schedule)
    for bi_, (c0, c1) in enumerate(MASK_BLOCKS):
        blocks.append((offs[c0] // 2, (offs[c1] - offs[c0]) // 2))
        for c in range(c0, c1):
            chunk_block[c] = bi_
    max_blk = max(n for _, n in blocks)

    nc = bacc.Bacc()
    stream = nc.dram_tensor("stream", [p * r * C], f8, kind="ExternalInput")
    # idxu holds local batch ids, even/odd-position split per mask block
    idxu = nc.dram_tensor("idxu", [p * r], u8, kind="ExternalInput")
    # psum/out keep 128 partitions: out partition dim = lhsT free dim = 128
    out = nc.dram_tensor("out", [2 * C, 2 * W], f32, kind="ExternalOutput")

    with TileContext(nc) as tc:
        with (
            tc.tile_pool(name="const", bufs=1) as cpool,
            tc.tile_pool(name="feats", bufs=fbufs) as fpool,
            tc.tile_pool(name="mask", bufs=mbufs) as mpool,
            tc.tile_pool(name="psum", bufs=1, space="PSUM") as ppool,
            tc.tile_pool(name="outp", bufs=1) as opool,
        ):
            # idx per-partition layout: per mask block [even pairs | odd pairs]
            idx_sb = cpool.tile([p, r], u8)
            idx_dram = idxu[:].rearrange("(p r) -> p r", p=p)
            split = 2 * blocks[0][1]  # bytes covering mask block 0 (even+odd)
            # idx rides the otherwise-idle gpsimd queue, block 0 first
            nc.gpsimd.dma_start(out=idx_sb[:, :split], in_=idx_dram[:, :split])
            nc.gpsimd.dma_start(out=idx_sb[:, split:], in_=idx_dram[:, split:])

            zero_sb = cpool.tile([p, 2 * C], f8)
            nc.vector.memset(zero_sb[:], 0.0)

            psum = ppool.tile([2 * C, 2 * W], f32)
            # zero-valued "start" matmul: all real matmuls then accumulate
            # (start=False), making the result independent of the
            # has_written-clear granularity
            nc.tensor.matmul(
                psum[:],
                lhsT=zero_sb[:],
                rhs=zero_sb[:, : 2 * W],
                start=True,
                stop=False,
            )

            # mask generation (DVE):
            # mask2[p, half*W+j, q] = (idx[p, half, start+q] == j)
            mask_tiles = {}

            def gen_masks(bi_):
                start, npair = blocks[bi_]
                mk = mpool.tile([p, 2 * W * max_blk], f8, tag="mk")
                mkv = mk[:, : 2 * W * npair].rearrange("p (e t) -> p e t", e=2 * W)
                for half in range(2):
                    base = 2 * start + half * npair
                    src = idx_sb[:, base : base + npair]
                    for j in range(W):
                        nc.vector.tensor_scalar(
                            out=mkv[:, half * W + j, :],
                            in0=src,
                            scalar1=float(j),
                            scalar2=None,
                            op0=mybir.AluOpType.is_equal,
                        )
                mask_tiles[bi_] = (mkv, start)

            gen_masks(0)
            k = 0
            for j, t in enumerate(schedule):
                if chunk_block[j] not in mask_tiles:
                    gen_masks(chunk_block[j])
                # look ahead one chunk so mask gen runs a block early
                if j + 1 < len(schedule) and chunk_block[j + 1] not in mask_tiles:
                    gen_masks(chunk_block[j + 1])
                ft = fpool.tile([p, t * C], f8, tag="ft")
                nc.scalar.dma_start(
                    out=ft[:],
                    in_=stream[p * offs[j] * C : p * offs[j + 1] * C].rearrange(
                        "(p x) -> p x", p=p
                    ),
                )
                mkv, mstart = mask_tiles[chunk_block[j]]
                for s in range(t // 2):
                    q = offs[j] // 2 + s  # global pair index
                    nc.tensor.matmul(
                        psum[:],
                        lhsT=ft[:, s * 2 * C : (s + 1) * 2 * C],
                        rhs=mkv[:, :, q - mstart],
                        start=False,
                        stop=(k == n_pair - 1),
                    )
                    k += 1
            out_sb = opool.tile([2 * C, 2 * W], f32)
            nc.vector.tensor_copy(out=out_sb[:], in_=psum[:])
            nc.gpsimd.dma_start(out=out[:, :], in_=out_sb[:])
    nc.finalize()
    return nc


def host_prep(feats, batch_idx):
    """Build per-core input maps (packed fp8 stream + local idx) from full inputs."""
    import ml_dtypes

    feats = np.asarray(feats, dtype=np.float32)
    bi = np.asarray(batch_idx).astype(np.int64)
    n, c = feats.shape
    assert n == N_TOTAL and c == C, (n, c)

    f8 = ml_dtypes.float8_e3m4
    q = feats.astype(f8)
    counts = np.bincount(bi, minlength=B).astype(np.float64)
    offs = np.concatenate([[0], np.cumsum(SCHEDULE)])
    # mask block boundaries in pair units
    blk_bounds = [(offs[c0] // 2, offs[c1] // 2) for c0, c1 in MASK_BLOCKS]

    in_maps = []
    lo_ws = []
    for m in range(N_CORES):
        sl = slice(m * N_CORE, (m + 1) * N_CORE)
        lo, hi = int(bi[m * N_CORE]), int(bi[(m + 1) * N_CORE - 1])
        assert hi - lo + 1 <= W, (m, lo, hi)
        lo_w = min(lo, B - W)
        lo_ws.append(lo_w)

        fpad = np.zeros((P * R, C), dtype=f8)
        fpad[:N_CORE] = q[sl]
        fv = fpad.reshape(P, R, C)
        ipad = np.full(P * R, PAD_IDX, dtype=np.uint8)
        ipad[:N_CORE] = (bi[sl] - lo_w).astype(np.uint8)
        # split even/odd positions per mask block: per partition, each
        # block stores [even pairs | odd pairs]
        iv = ipad.reshape(P, R // 2, 2)
        iparts = []
        for q0, q1 in blk_bounds:
            blk = iv[:, q0:q1]  # [P, npair, 2]
            iparts.append(np.ascontiguousarray(blk.transpose(0, 2, 1)))
        iflat = np.concatenate([x.reshape(P, -1) for x in iparts], axis=1)
        assert iflat.shape == (P, R)

        # chunk-major flat layout: chunk j = [p, t_j, C] contiguous block
        flat = np.empty(P * R * C, dtype=f8)
        pos = 0
        for j, t in enumerate(SCHEDULE):
            blk = fv[:, offs[j] : offs[j] + t]  # [P, t, C]
            flat[pos : pos + blk.size] = blk.reshape(-1)
            pos += blk.size
        in_maps.append({"stream": flat, "idxu": iflat.reshape(-1)})
    return in_maps, (lo_ws, counts)


_CACHED_NC = None


def get_program():
    global _CACHED_NC
    if _CACHED_NC is None:
        _CACHED_NC = build_program()
    return _CACHED_NC


def run_on_cores(in_maps, trace=False):
    _ensure_import_path()
    from concourse.bass_utils import run_bass_kernel_spmd

    nc = get_program()
    res = run_bass_kernel_spmd(nc, in_maps, list(range(N_CORES)), trace=trace)
    return res


def finalize(per_core_outs, aux):
    lo_ws, counts = aux
    sums = np.zeros((B, C), dtype=np.float64)
    for o, lo_w in zip(per_core_outs, lo_ws):
        o = np.asarray(o, dtype=np.float64)  # [128, 16]
        strip = o[:C, :W] + o[C:, W:]  # [64, 8] = sums^T (even + odd)
        sums[lo_w : lo_w + W] += strip.T
    pooled = sums / np.maximum(counts, 1.0)[:, None]
    return pooled.astype(np.float32)


def kernel(feats, batch_idx, num_batches):
    assert int(num_batches) == B
    in_maps, aux = host_prep(feats, batch_idx)
    res = run_on_cores(in_maps)
    return finalize([r["out"] for r in res.results], aux)
